# revision 24
# baseline (speedup 1.0000x reference)
"""Fused GroupNorm + multi-head self-attention + output projection for
nn_Attention_55619826483814 on 8 TRN2 NeuronCores.

Reference computation (shapes hardcoded):
  x: (4, 256, 64, 64) f32
  GroupNorm(1 group) over (C,H,W) per sample -> per-channel affine (gamma, beta)
  qkv = w_qkv @ xn  (384 = 3*4heads*32dim rows)
  per head: sim = (q*scale)^T k ; attn = softmax(sim, axis=j) ; out = attn @ v
  y = w_out @ out + b_out     -> (4, 256, 64, 64)

Sharding: 8 cores = 4 batches x 2 spatial halves (identical to the previous
baseline). Core c handles batch c//2 and query positions [2048*(c%2), +2048),
with the spatial axis rolled host-side so its query half sits at columns
0..2047. Keys/values/groupnorm stats use all 4096 positions.

On-device design (v2) - the kernel is softmax-throughput bound, so both
pointwise engines stream exp concurrently while the PE runs packed matmuls:

  sim:  4 heads packed in the 128x128 PE via 4x row tiling
        (tile_position=(32h,0), K=32 each) -> one (128,512) f32 bank per
        unit (jc, h); units paired into (128,1024) PSUM slots.
  exp:  each slot is consumed by EITHER ScalarE (exact exp, scale folded)
        OR VectorE (Schraudolph: int16 <- round(sim*C + D), bitcast bf16;
        ~1.8% rms error on e^x), statically interleaved ~51/49 so both
        engines stay saturated. That beats the single-engine ACT floor
        (~295us in the baseline) by ~2x.
  attv: 4 heads packed via 4x column tiling (tile_position=(0,32h), M=32)
        accumulating U[32h:32h+32] over jc in one persistent bank;
        denominators via M=1 col-tiled ones-matmuls into a second bank.
  tail: denominator broadcast via per-quadrant indicator matmuls,
        reciprocal_approx_fast, U*(1/d) -> bf16, w_out projection, bias.

PSUM: 6 banks = 3 rotating (128,1024) slots, 1 bank U_t, 1 bank den_t.
"""

import sys

sys.path.insert(0, "/opt/trn_rl_repo")

import numpy as np

import concourse.bass as bass
import concourse.mybir as mybir
import concourse.tile as tile
from concourse import bacc
from concourse.masks import make_identity

DT = mybir.dt
F32 = DT.float32
BF16 = DT.bfloat16
I16 = DT.int16
ALU = mybir.AluOpType
ACTF = mybir.ActivationFunctionType

DIM = 256  # channels
N = 4096  # spatial positions
NH = 2048  # per-core query half
HEADS = 4
DH = 32  # head dim
HID = 128
SCALE = DH ** -0.5
EPS = 1e-5
NTOT = DIM * N

N_CORES = 8

LOG2E = 1.4426950408889634
C_SCH = 128.0 * LOG2E * SCALE  # Schraudolph scale (attn scale folded in)
D_SCH = 127.0 * 128.0 - 7.5  # bias; -7.5 centers the exp ratio error

# Of the 64 slots per i-tile, how many go to ScalarE (rest to VectorE).
ACT_SLOTS_PER_T = 36

import os
KSTAGE = os.environ.get("KSTAGE", "full")  # full | noattv | notail | nodp


def build_nc():
    nc = bacc.Bacc("TRN2", target_bir_lowering=False)

    xr_d = nc.dram_tensor("xr", [DIM, N], F32, kind="ExternalInput")
    wq_d = nc.dram_tensor("wq", [3 * HID, DIM], F32, kind="ExternalInput")
    wo_d = nc.dram_tensor("wo", [DIM, HID], F32, kind="ExternalInput")
    bo_d = nc.dram_tensor("bo", [DIM, 1], F32, kind="ExternalInput")
    gam_d = nc.dram_tensor("gam", [DIM, 1], F32, kind="ExternalInput")
    bet_d = nc.dram_tensor("bet", [DIM, 1], F32, kind="ExternalInput")
    y_d = nc.dram_tensor("y", [DIM, NH], F32, kind="ExternalOutput")

    with tile.TileContext(nc) as tc:
        with (
            tc.tile_pool(name="small", bufs=1) as small,
            tc.tile_pool(name="big", bufs=1) as big,
            tc.tile_pool(name="pxf", bufs=1) as pxf,
            tc.tile_pool(name="pjunk", bufs=1) as pjunk,
            tc.tile_pool(name="pwst", bufs=3) as pwst,
            tc.tile_pool(name="ptiny", bufs=2) as ptiny,
            tc.tile_pool(name="pexa", bufs=6) as pexa,
            tc.tile_pool(name="pexd", bufs=6) as pexd,
            tc.tile_pool(name="ptail", bufs=2) as ptail,
            tc.tile_pool(name="slots", bufs=3, space="PSUM") as slots,
            tc.tile_pool(name="pu", bufs=1, space="PSUM") as pu,
            tc.tile_pool(name="pden", bufs=1, space="PSUM") as pden,
        ):
            # ---------- constants ----------
            identity = small.tile([128, 128], F32, tag="ident")
            make_identity(nc, identity[:])

            ones128x32 = small.tile([128, 32], F32, tag="o12832")
            nc.gpsimd.memset(ones128x32[:], 1.0)
            ones1x128 = small.tile([1, 128], F32, tag="o1128")
            nc.gpsimd.memset(ones1x128[:], 1.0)
            ones128x1 = small.tile([128, 1], BF16, tag="o1281")
            nc.gpsimd.memset(ones128x1[:], 1.0)
            # e_full[32h, c] = (c//32 == h), used to broadcast den rows
            e_full = small.tile([128, 128], BF16, tag="efull")
            nc.gpsimd.memset(e_full[:], 0.0)
            for h in range(HEADS):
                nc.gpsimd.memset(
                    e_full[32 * h : 32 * h + 1, 32 * h : 32 * h + 32], 1.0
                )

            # ---------- load x FIRST (sync queue issues DMAs serially at
            # ~0.65us each, so x must be at the head of the queue) ----------
            xf = []
            for kc in range(2):
                t = pxf.tile([128, N], F32, tag=f"xf{kc}", name=f"xf{kc}")
                for p in range(4):
                    nc.sync.dma_start(
                        t[:, 1024 * p : 1024 * p + 1024],
                        xr_d[128 * kc : 128 * kc + 128, 1024 * p : 1024 * p + 1024])
                xf.append(t)

            gam_c, bet_c, bo_c = [], [], []
            for kc in range(2):
                g = small.tile([128, 1], F32, tag=f"gam{kc}", name=f"gam{kc}")
                nc.sync.dma_start(g[:], gam_d[128 * kc : 128 * kc + 128, :])
                gam_c.append(g)
                bt = small.tile([128, 1], F32, tag=f"bet{kc}", name=f"bet{kc}")
                nc.sync.dma_start(bt[:], bet_d[128 * kc : 128 * kc + 128, :])
                bet_c.append(bt)
                bb = small.tile([128, 1], F32, tag=f"bo{kc}", name=f"bo{kc}")
                nc.sync.dma_start(bb[:], bo_d[128 * kc : 128 * kc + 128, :])
                bo_c.append(bb)

            # ---------- weight transposes ----------
            wqkvT = [big.tile([128, 384], BF16, tag=f"wqkvT{c}", name=f"wqkvT{c}") for c in range(2)]
            for r in range(3):
                wst = pwst.tile([128, DIM], F32, tag="wst")
                nc.sync.dma_start(wst[:], wq_d[128 * r : 128 * r + 128, :])
                for c in range(2):
                    tp = slots.tile([128, 1024], F32, tag="slot", name=f"wqt{r}{c}")
                    nc.tensor.transpose(tp[:, 0:128], wst[:, 128 * c : 128 * c + 128], identity[:])
                    nc.vector.tensor_copy(wqkvT[c][:, 128 * r : 128 * r + 128], tp[:, 0:128])
            woT = big.tile([128, DIM], BF16, tag="woT")
            for r in range(2):
                wst = pwst.tile([128, HID], F32, tag="wst")
                nc.sync.dma_start(wst[:], wo_d[128 * r : 128 * r + 128, :])
                tp = slots.tile([128, 1024], F32, tag="slot", name=f"wot{r}")
                nc.tensor.transpose(tp[:, 0:128], wst[:], identity[:])
                nc.vector.tensor_copy(woT[:, 128 * r : 128 * r + 128], tp[:, 0:128])

            # ---------- groupnorm stats ----------
            # per-piece partial sums: cols 0-7 = sum(x), cols 8-15 = sum(x^2)
            st = ptiny.tile([128, 16], F32, tag="st")
            for kc in range(2):
                for p in range(4):
                    piece = xf[kc][:, 1024 * p : 1024 * p + 1024]
                    col = 4 * kc + p
                    nc.vector.reduce_sum(st[:, col : col + 1], piece,
                                         axis=mybir.AxisListType.X)
                    junk = pjunk.tile([128, 1024], BF16, tag="junk")
                    nc.scalar.activation(junk[:], piece, ACTF.Square,
                                         accum_out=st[:, 8 + col : 9 + col])
            stp = pu.tile([128, 512], F32, tag="u")
            nc.tensor.matmul(stp[0:32, 0:16], lhsT=ones128x32[:], rhs=st[:], start=True, stop=True)

            stp_sb = ptiny.tile([1, 16], F32, tag="stpsb")
            nc.vector.tensor_copy(stp_sb[:], stp[0:1, 0:16])
            mm = ptiny.tile([1, 2], F32, tag="mm")
            nc.vector.reduce_sum(mm[:], stp_sb[:].rearrange("p (a b) -> p a b", a=2),
                                 axis=mybir.AxisListType.X)
            mm2 = ptiny.tile([1, 2], F32, tag="mm2")
            nc.vector.tensor_scalar_mul(mm2[:], mm[:], 1.0 / NTOT)  # [mu, E[x^2]]
            musq = ptiny.tile([1, 1], F32, tag="musq")
            nc.vector.tensor_tensor(musq[:], mm2[:, 0:1], mm2[:, 0:1], ALU.mult)
            var = ptiny.tile([1, 1], F32, tag="var")
            nc.vector.tensor_tensor(var[:], mm2[:, 1:2], musq[:], ALU.subtract)
            vare = ptiny.tile([1, 1], F32, tag="vare")
            nc.vector.tensor_scalar_add(vare[:], var[:], EPS)
            sq = ptiny.tile([1, 1], F32, tag="sq")
            nc.scalar.activation(sq[:], vare[:], ACTF.Sqrt)
            s_t = ptiny.tile([1, 1], F32, tag="s")
            nc.vector.reciprocal(s_t[:], sq[:])
            sm = ptiny.tile([1, 2], F32, tag="sm")
            nc.vector.tensor_copy(sm[:, 0:1], s_t[:])
            nc.vector.tensor_copy(sm[:, 1:2], mm2[:, 0:1])
            bsm = pden.tile([128, 512], F32, tag="den")
            nc.tensor.matmul(bsm[:, 0:2], lhsT=ones1x128[:], rhs=sm[:], start=True, stop=True)
            a_c, d_c = [], []
            for kc in range(2):
                a = ptiny.tile([128, 1], F32, tag="ac", name=f"ac{kc}")
                nc.vector.tensor_tensor(a[:], gam_c[kc][:], bsm[:, 0:1], ALU.mult)
                a_c.append(a)
                t1 = ptiny.tile([128, 1], F32, tag="t1", name=f"t1{kc}")
                nc.vector.tensor_tensor(t1[:], a[:], bsm[:, 1:2], ALU.mult)
                d = ptiny.tile([128, 1], F32, tag="dc", name=f"dc{kc}")
                nc.vector.tensor_tensor(d[:], bet_c[kc][:], t1[:], ALU.subtract)
                d_c.append(d)

            # force the ACT exp table load now (overlaps the qkv phase)
            dummy = ptiny.tile([1, 1], F32, tag="dummy")
            nc.scalar.activation(dummy[:], vare[:], ACTF.Exp)

            # ---------- normalize + cast ----------
            xn3 = big.tile([128, 2, N], BF16, tag="xn")
            for kc in range(2):
                nc.vector.tensor_scalar(xn3[:, kc, :], xf[kc][:], a_c[kc][:],
                                        d_c[kc][:], ALU.mult, ALU.add)

            # HAM warmup: dense no-reload burst gated on xn (same gate as
            # the first qkv matmul) so the PE enters the qkv+attention stream
            # at full clock. ldweights=False keeps the array ~100% busy.
            warm = pu.tile([128, 512], F32, tag="u", name="warm")
            for w in range(12):
                wm = nc.tensor.matmul(warm[:], lhsT=wqkvT[0][:, 0:128],
                                      rhs=xn3[:, 0, 0:512],
                                      start=True, stop=True)
                if w > 0:
                    wm.ins.ldweights = False

            # ---------- qkv projections ----------
            # K/Q packed (128, n): head h lives at partitions 32h..32h+31
            # (q/k channel-major == head-major). evictions alternate DVE/ACT.
            Kp = big.tile([128, N], BF16, tag="Kp")
            Qp = big.tile([128, NH], BF16, tag="Qp")
            vt3 = big.tile([128, 32, 128], BF16, tag="vt3")
            evict_i = 0

            def evict(dst_ap, src_ap):
                nonlocal evict_i
                if evict_i % 2 == 0:
                    nc.vector.tensor_copy(dst_ap, src_ap)
                else:
                    nc.scalar.activation(dst_ap, src_ap, ACTF.Copy)
                evict_i += 1

            for g in range(4):  # K: 2 nt of 512 per slot-tile
                pk = slots.tile([128, 1024], F32, tag="slot", name=f"kp{g}")
                for half in range(2):
                    nt = 2 * g + half
                    for kc in range(2):
                        nc.tensor.matmul(pk[:, 512 * half : 512 * half + 512],
                                         lhsT=wqkvT[kc][:, 128:256],
                                         rhs=xn3[:, kc, 512 * nt : 512 * nt + 512],
                                         start=(kc == 0), stop=(kc == 1))
                evict(Kp[:, 1024 * g : 1024 * g + 1024], pk[:])
            for g in range(2):  # Q: raw q, no scale (folded into exp)
                pq = slots.tile([128, 1024], F32, tag="slot", name=f"qp{g}")
                for half in range(2):
                    nt = 2 * g + half
                    for kc in range(2):
                        nc.tensor.matmul(pq[:, 512 * half : 512 * half + 512],
                                         lhsT=wqkvT[kc][:, 0:128],
                                         rhs=xn3[:, kc, 512 * nt : 512 * nt + 512],
                                         start=(kc == 0), stop=(kc == 1))
                evict(Qp[:, 1024 * g : 1024 * g + 1024], pq[:])
            for g in range(4):  # V^T: 8 jt of (128,128) per slot-tile
                pv = slots.tile([128, 1024], F32, tag="slot", name=f"vp{g}")
                for half in range(8):
                    jt = 8 * g + half
                    for kc in range(2):
                        nc.tensor.matmul(pv[:, 128 * half : 128 * half + 128],
                                         lhsT=xn3[:, kc, 128 * jt : 128 * jt + 128],
                                         rhs=wqkvT[kc][:, 256:384],
                                         start=(kc == 0), stop=(kc == 1))
                evict(vt3[:, 8 * g : 8 * g + 8, :],
                      pv[:].rearrange("p (a b) -> p a b", a=8))

            # ---------- attention ----------
            # flat stream over 256 slots (t = s//64); attv/den pops lag by
            # ~2-3 slots and cross i-tile boundaries so the PE never drains
            # at a boundary (HAM stays warm); tail(t) is emitted as soon as
            # t's last slot is popped, interleaved with t+1's sim stream.
            def act_slot(sl):
                return (sl * ACT_SLOTS_PER_T) // 64 != ((sl + 1) * ACT_SLOTS_PER_T) // 64

            U_t, den_t = {}, {}

            def get_acc(t):
                if t not in U_t:
                    U_t[t] = pu.tile([128, 512], F32, tag="u", name=f"U{t}")
                    den_t[t] = pden.tile([128, 512], F32, tag="den",
                                         name=f"den{t}")
                return U_t[t], den_t[t]

            def attv(ex_bf, t, units):
                Ut, dent = get_acc(t)
                for q, (jc, h) in enumerate(units):
                    nc.tensor.matmul(
                        Ut[32 * h : 32 * h + 32, :],
                        lhsT=vt3[:, jc, 32 * h : 32 * h + 32],
                        rhs=ex_bf[:, 512 * q : 512 * q + 512],
                        start=(jc == 0), stop=(jc == 31),
                        tile_position=(0, 32 * h),
                        skip_group_check=True)
                for q, (jc, h) in enumerate(units):
                    nc.tensor.matmul(
                        dent[32 * h : 32 * h + 1, :],
                        lhsT=ones128x1[:],
                        rhs=ex_bf[:, 512 * q : 512 * q + 512],
                        start=(jc == 0), stop=(jc == 31),
                        tile_position=(0, 32 * h),
                        skip_group_check=True)

            attnT_t = {}

            def tail_a(t):
                # normalize chain only: frees U (for U_{t+1}) as early as
                # possible and keeps the slot ring untouched. High priority:
                # the scheduler slots these ahead of queued independent exps,
                # shortening the U_t -> U_{t+1} handover.
                Ut, dent = U_t[t], den_t[t]
                ctx = tc.high_priority(offset=40)
                ctx.__enter__()
                attnT = ptail.tile([128, 512], BF16, tag="attnT", name=f"at{t}")
                den_sb = ptail.tile([128, 512], BF16, tag="densb", name=f"dsb{t}")
                nc.scalar.activation(den_sb[:], dent[:], ACTF.Copy)
                # dp reuses the den bank (freed by the copy above)
                dp = pden.tile([128, 512], F32, tag="den", name=f"dp{t}")
                # e_full rows 32h pick den_sb row 32h and broadcast to
                # partitions 32h..32h+31 (all other rows are zero)
                nc.tensor.matmul(dp[:], lhsT=e_full[:],
                                 rhs=den_sb[:], start=True, stop=True)
                rsb = ptail.tile([128, 512], F32, tag="rsb", name=f"rsb{t}")
                nc.vector.reciprocal_approx_fast(rsb[:], dp[:])
                nc.vector.tensor_tensor(attnT[:], Ut[:], rsb[:], ALU.mult)
                ctx.__exit__(None, None, None)
                attnT_t[t] = attnT

            def tail_b(t):
                # projection + bias + store; emitted a few slots later so the
                # py matmuls never head-block the PE queue on the DVE chain.
                attnT = attnT_t[t]
                py = slots.tile([128, 1024], F32, tag="slot", name=f"py{t}")
                for kc in range(2):
                    nc.tensor.matmul(py[:, 512 * kc : 512 * kc + 512],
                                     lhsT=woT[:, 128 * kc : 128 * kc + 128],
                                     rhs=attnT[:], start=True, stop=True)
                for kc in range(2):
                    ysb = ptail.tile([128, 512], F32, tag="ysb", name=f"y{t}{kc}")
                    if kc == 0:
                        nc.scalar.activation(ysb[:], py[:, 512 * kc : 512 * kc + 512],
                                             ACTF.Identity, bias=bo_c[0][:])
                    else:
                        nc.vector.tensor_scalar_add(ysb[:], py[:, 512 * kc : 512 * kc + 512],
                                                    bo_c[1][:])
                    nc.sync.dma_start(
                        y_d[128 * kc : 128 * kc + 128, 512 * t : 512 * t + 512],
                        ysb[:])

            pending = []
            popped = [0, 0, 0, 0]
            tb_done = [False] * 4

            def pop_one():
                ex_bf, pt, punits = pending.pop(0)
                attv(ex_bf, pt, punits)
                popped[pt] += len(punits)
                if popped[pt] == 128:
                    tail_a(pt)
                if pt > 0 and popped[pt] >= 12 and not tb_done[pt - 1]:
                    tb_done[pt - 1] = True
                    tail_b(pt - 1)

            for t in range(4):
                units_all = [(jc, h) for jc in range(32) for h in range(HEADS)]
                slot_units = [units_all[i : i + 2]
                              for i in range(0, 128, 2)]
                for sl, units in enumerate(slot_units):
                    w = 512 * len(units)
                    sp = slots.tile([128, 1024], F32, tag="slot",
                                    name=f"s{t}_{sl}")
                    simctx = tc.high_priority(offset=30)
                    simctx.__enter__()
                    for q, (jc, h) in enumerate(units):
                        nc.tensor.matmul(
                            sp[:, 512 * q : 512 * q + 512],
                            lhsT=Kp[32 * h : 32 * h + 32,
                                    128 * jc : 128 * jc + 128],
                            rhs=Qp[32 * h : 32 * h + 32,
                                   512 * t : 512 * t + 512],
                            start=True, stop=True,
                            tile_position=(32 * h, 0))
                    simctx.__exit__(None, None, None)
                    if act_slot(sl):
                        ex = pexa.tile([128, 1024], BF16, tag="exa")
                        nc.scalar.activation(ex[:, 0:w], sp[:, 0:w], ACTF.Exp,
                                             scale=SCALE)
                        ex_bf = ex[:]
                    else:
                        exd = pexd.tile([128, 1024], I16, tag="exd")
                        nc.vector.tensor_scalar(exd[:, 0:w], sp[:, 0:w],
                                                C_SCH, D_SCH,
                                                ALU.mult, ALU.add)
                        ex_bf = exd[:].bitcast(BF16)
                    pending.append((ex_bf, t, units))
                    if sl % 2 == 1 and len(pending) >= 6:
                        pop_one()
                        pop_one()
            while pending:
                pop_one()
            tail_b(3)

    nc.compile()
    return nc


_NC_CACHE = None


def get_nc():
    global _NC_CACHE
    if _NC_CACHE is None:
        _NC_CACHE = build_nc()
    return _NC_CACHE


def shard_inputs(x, gamma, beta, w_qkv, w_out, b_out):
    """Build the 8 per-core input maps (pure slicing / layout, no math)."""
    x = np.ascontiguousarray(np.asarray(x, dtype=np.float32))
    b, c, hh, ww = x.shape
    assert (b, c, hh, ww) == (4, DIM, 64, 64)
    xf = x.reshape(b, DIM, N)
    wq = np.ascontiguousarray(np.asarray(w_qkv, dtype=np.float32))
    wo = np.ascontiguousarray(np.asarray(w_out, dtype=np.float32))
    bo = np.asarray(b_out, dtype=np.float32).reshape(DIM, 1)
    gam = np.asarray(gamma, dtype=np.float32).reshape(DIM, 1)
    bet = np.asarray(beta, dtype=np.float32).reshape(DIM, 1)
    in_maps = []
    for core in range(N_CORES):
        bi, half = core // 2, core % 2
        xr = xf[bi] if half == 0 else np.roll(xf[bi], -NH, axis=1)
        in_maps.append({
            "xr": np.ascontiguousarray(xr),
            "wq": wq, "wo": wo, "bo": bo, "gam": gam, "bet": bet,
        })
    return in_maps


def gather_outputs(per_core_y):
    """per_core_y: list of 8 arrays (256, 2048) -> (4, 256, 64, 64) f32."""
    y = np.empty((4, DIM, N), dtype=np.float32)
    for core in range(N_CORES):
        bi, half = core // 2, core % 2
        y[bi][:, NH * half : NH * half + NH] = per_core_y[core]
    return y.reshape(4, DIM, 64, 64)


def kernel(x, gamma, beta, w_qkv, w_out, b_out):
    from concourse.bass_utils import run_bass_kernel_spmd

    nc = get_nc()
    in_maps = shard_inputs(x, gamma, beta, w_qkv, w_out, b_out)
    res = run_bass_kernel_spmd(nc, in_maps, core_ids=list(range(N_CORES)))
    return gather_outputs([res.results[c]["y"] for c in range(N_CORES)])


# revision 26
# speedup vs baseline: 1.2247x; 1.2247x over previous
"""Fused GroupNorm + multi-head self-attention + output projection for
nn_Attention_55619826483814 on 8 TRN2 NeuronCores.

Reference computation (shapes hardcoded):
  x: (4, 256, 64, 64) f32
  GroupNorm(1 group) over (C,H,W) per sample -> per-channel affine (gamma, beta)
  qkv = w_qkv @ xn  (384 = 3*4heads*32dim rows)
  per head: sim = (q*scale)^T k ; attn = softmax(sim, axis=j) ; out = attn @ v
  y = w_out @ out + b_out     -> (4, 256, 64, 64)

Sharding: 8 cores = 4 batches x 2 spatial halves (identical to the previous
baseline). Core c handles batch c//2 and query positions [2048*(c%2), +2048),
with the spatial axis rolled host-side so its query half sits at columns
0..2047. Keys/values/groupnorm stats use all 4096 positions.

On-device design (v2) - the kernel is softmax-throughput bound, so both
pointwise engines stream exp concurrently while the PE runs packed matmuls:

  sim:  4 heads packed in the 128x128 PE via 4x row tiling
        (tile_position=(32h,0), K=32 each) -> one (128,512) f32 bank per
        unit (jc, h); units paired into (128,1024) PSUM slots.
  exp:  each slot is consumed by EITHER ScalarE (exact exp, scale folded)
        OR VectorE (Schraudolph: int16 <- round(sim*C + D), bitcast bf16;
        ~1.8% rms error on e^x), statically interleaved ~51/49 so both
        engines stay saturated. That beats the single-engine ACT floor
        (~295us in the baseline) by ~2x.
  attv: 4 heads packed via 4x column tiling (tile_position=(0,32h), M=32)
        accumulating U[32h:32h+32] over jc in one persistent bank;
        denominators via M=1 col-tiled ones-matmuls into a second bank.
  tail: denominator broadcast via per-quadrant indicator matmuls,
        reciprocal_approx_fast, U*(1/d) -> bf16, w_out projection, bias.

PSUM: 6 banks = 3 rotating (128,1024) slots, 1 bank U_t, 1 bank den_t.
"""

import sys

sys.path.insert(0, "/opt/trn_rl_repo")

import numpy as np

import concourse.bass as bass
import concourse.mybir as mybir
import concourse.tile as tile
from concourse import bacc
from concourse.masks import make_identity

DT = mybir.dt
F32 = DT.float32
BF16 = DT.bfloat16
I16 = DT.int16
ALU = mybir.AluOpType
ACTF = mybir.ActivationFunctionType

DIM = 256  # channels
N = 4096  # spatial positions
NH = 2048  # per-core query half
HEADS = 4
DH = 32  # head dim
HID = 128
SCALE = DH ** -0.5
EPS = 1e-5
NTOT = DIM * N

N_CORES = 8

LOG2E = 1.4426950408889634
C_SCH = 128.0 * LOG2E * SCALE  # Schraudolph scale (attn scale folded in)
D_SCH = 127.0 * 128.0 - 7.5  # bias; -7.5 centers the exp ratio error

# Of the 64 slots per i-tile, how many go to ScalarE (rest to VectorE).
ACT_SLOTS_PER_T = 36


def build_nc():
    nc = bacc.Bacc("TRN2", target_bir_lowering=False)

    xr_d = nc.dram_tensor("xr", [DIM, N], F32, kind="ExternalInput")
    wq_d = nc.dram_tensor("wq", [3 * HID, DIM], F32, kind="ExternalInput")
    wo_d = nc.dram_tensor("wo", [DIM, HID], F32, kind="ExternalInput")
    bo_d = nc.dram_tensor("bo", [DIM, 1], F32, kind="ExternalInput")
    gam_d = nc.dram_tensor("gam", [DIM, 1], F32, kind="ExternalInput")
    bet_d = nc.dram_tensor("bet", [DIM, 1], F32, kind="ExternalInput")
    y_d = nc.dram_tensor("y", [DIM, NH], F32, kind="ExternalOutput")

    with tile.TileContext(nc) as tc:
        with (
            tc.tile_pool(name="small", bufs=1) as small,
            tc.tile_pool(name="big", bufs=1) as big,
            tc.tile_pool(name="pxf", bufs=1) as pxf,
            tc.tile_pool(name="pjunk", bufs=1) as pjunk,
            tc.tile_pool(name="pwst", bufs=3) as pwst,
            tc.tile_pool(name="ptiny", bufs=2) as ptiny,
            tc.tile_pool(name="pexa", bufs=6) as pexa,
            tc.tile_pool(name="pexd", bufs=6) as pexd,
            tc.tile_pool(name="ptail", bufs=2) as ptail,
            tc.tile_pool(name="slots", bufs=3, space="PSUM") as slots,
            tc.tile_pool(name="pu", bufs=1, space="PSUM") as pu,
            tc.tile_pool(name="pden", bufs=1, space="PSUM") as pden,
        ):
            # ---------- constants ----------
            identity = small.tile([128, 128], F32, tag="ident")
            make_identity(nc, identity[:])

            ones128x32 = small.tile([128, 32], F32, tag="o12832")
            nc.gpsimd.memset(ones128x32[:], 1.0)
            ones1x128 = small.tile([1, 128], F32, tag="o1128")
            nc.gpsimd.memset(ones1x128[:], 1.0)
            ones128x1 = small.tile([128, 1], BF16, tag="o1281")
            nc.gpsimd.memset(ones128x1[:], 1.0)
            # e_full[32h, c] = (c//32 == h), used to broadcast den rows
            e_full = small.tile([128, 128], BF16, tag="efull")
            nc.gpsimd.memset(e_full[:], 0.0)
            for h in range(HEADS):
                nc.gpsimd.memset(
                    e_full[32 * h : 32 * h + 1, 32 * h : 32 * h + 32], 1.0
                )

            # ---------- load x FIRST (sync queue issues DMAs serially at
            # ~0.65us each, so x must be at the head of the queue) ----------
            xf = []
            for kc in range(2):
                t = pxf.tile([128, N], F32, tag=f"xf{kc}", name=f"xf{kc}")
                for p in range(4):
                    nc.sync.dma_start(
                        t[:, 1024 * p : 1024 * p + 1024],
                        xr_d[128 * kc : 128 * kc + 128, 1024 * p : 1024 * p + 1024])
                xf.append(t)

            gam_c, bet_c, bo_c = [], [], []
            for kc in range(2):
                g = small.tile([128, 1], F32, tag=f"gam{kc}", name=f"gam{kc}")
                nc.sync.dma_start(g[:], gam_d[128 * kc : 128 * kc + 128, :])
                gam_c.append(g)
                bt = small.tile([128, 1], F32, tag=f"bet{kc}", name=f"bet{kc}")
                nc.sync.dma_start(bt[:], bet_d[128 * kc : 128 * kc + 128, :])
                bet_c.append(bt)
                bb = small.tile([128, 1], F32, tag=f"bo{kc}", name=f"bo{kc}")
                nc.sync.dma_start(bb[:], bo_d[128 * kc : 128 * kc + 128, :])
                bo_c.append(bb)

            # ---------- weight transposes ----------
            wqkvT = [big.tile([128, 384], BF16, tag=f"wqkvT{c}", name=f"wqkvT{c}") for c in range(2)]
            for r in range(3):
                wst = pwst.tile([128, DIM], F32, tag="wst")
                nc.sync.dma_start(wst[:], wq_d[128 * r : 128 * r + 128, :])
                for c in range(2):
                    tp = slots.tile([128, 1024], F32, tag="slot", name=f"wqt{r}{c}")
                    nc.tensor.transpose(tp[:, 0:128], wst[:, 128 * c : 128 * c + 128], identity[:])
                    nc.vector.tensor_copy(wqkvT[c][:, 128 * r : 128 * r + 128], tp[:, 0:128])
            woT = big.tile([128, DIM], BF16, tag="woT")
            for r in range(2):
                wst = pwst.tile([128, HID], F32, tag="wst")
                nc.sync.dma_start(wst[:], wo_d[128 * r : 128 * r + 128, :])
                tp = slots.tile([128, 1024], F32, tag="slot", name=f"wot{r}")
                nc.tensor.transpose(tp[:, 0:128], wst[:], identity[:])
                nc.vector.tensor_copy(woT[:, 128 * r : 128 * r + 128], tp[:, 0:128])

            # ---------- groupnorm stats ----------
            # per-piece partial sums: cols 0-7 = sum(x), cols 8-15 = sum(x^2)
            st = ptiny.tile([128, 16], F32, tag="st")
            for kc in range(2):
                for p in range(4):
                    piece = xf[kc][:, 1024 * p : 1024 * p + 1024]
                    col = 4 * kc + p
                    nc.vector.reduce_sum(st[:, col : col + 1], piece,
                                         axis=mybir.AxisListType.X)
                    junk = pjunk.tile([128, 1024], BF16, tag="junk")
                    nc.scalar.activation(junk[:], piece, ACTF.Square,
                                         accum_out=st[:, 8 + col : 9 + col])
            stp = pu.tile([128, 512], F32, tag="u")
            nc.tensor.matmul(stp[0:32, 0:16], lhsT=ones128x32[:], rhs=st[:], start=True, stop=True)

            stp_sb = ptiny.tile([1, 16], F32, tag="stpsb")
            nc.vector.tensor_copy(stp_sb[:], stp[0:1, 0:16])
            mm = ptiny.tile([1, 2], F32, tag="mm")
            nc.vector.reduce_sum(mm[:], stp_sb[:].rearrange("p (a b) -> p a b", a=2),
                                 axis=mybir.AxisListType.X)
            mm2 = ptiny.tile([1, 2], F32, tag="mm2")
            nc.vector.tensor_scalar_mul(mm2[:], mm[:], 1.0 / NTOT)  # [mu, E[x^2]]
            musq = ptiny.tile([1, 1], F32, tag="musq")
            nc.vector.tensor_tensor(musq[:], mm2[:, 0:1], mm2[:, 0:1], ALU.mult)
            var = ptiny.tile([1, 1], F32, tag="var")
            nc.vector.tensor_tensor(var[:], mm2[:, 1:2], musq[:], ALU.subtract)
            vare = ptiny.tile([1, 1], F32, tag="vare")
            nc.vector.tensor_scalar_add(vare[:], var[:], EPS)
            sq = ptiny.tile([1, 1], F32, tag="sq")
            nc.scalar.activation(sq[:], vare[:], ACTF.Sqrt)
            s_t = ptiny.tile([1, 1], F32, tag="s")
            nc.vector.reciprocal(s_t[:], sq[:])
            sm = ptiny.tile([1, 2], F32, tag="sm")
            nc.vector.tensor_copy(sm[:, 0:1], s_t[:])
            nc.vector.tensor_copy(sm[:, 1:2], mm2[:, 0:1])
            bsm = pden.tile([128, 512], F32, tag="den")
            nc.tensor.matmul(bsm[:, 0:2], lhsT=ones1x128[:], rhs=sm[:], start=True, stop=True)
            a_c, d_c = [], []
            for kc in range(2):
                a = ptiny.tile([128, 1], F32, tag="ac", name=f"ac{kc}")
                nc.vector.tensor_tensor(a[:], gam_c[kc][:], bsm[:, 0:1], ALU.mult)
                a_c.append(a)
                t1 = ptiny.tile([128, 1], F32, tag="t1", name=f"t1{kc}")
                nc.vector.tensor_tensor(t1[:], a[:], bsm[:, 1:2], ALU.mult)
                d = ptiny.tile([128, 1], F32, tag="dc", name=f"dc{kc}")
                nc.vector.tensor_tensor(d[:], bet_c[kc][:], t1[:], ALU.subtract)
                d_c.append(d)

            # force the ACT exp table load now (overlaps the qkv phase)
            dummy = ptiny.tile([1, 1], F32, tag="dummy")
            nc.scalar.activation(dummy[:], vare[:], ACTF.Exp)

            # ---------- normalize + cast ----------
            xn3 = big.tile([128, 2, N], BF16, tag="xn")
            for kc in range(2):
                nc.vector.tensor_scalar(xn3[:, kc, :], xf[kc][:], a_c[kc][:],
                                        d_c[kc][:], ALU.mult, ALU.add)

            # HAM warmup: dense no-reload burst gated on xn (same gate as
            # the first qkv matmul) so the PE enters the qkv+attention stream
            # at full clock. ldweights=False keeps the array ~100% busy.
            warm = pu.tile([128, 512], F32, tag="u", name="warm")
            for w in range(12):
                wm = nc.tensor.matmul(warm[:], lhsT=wqkvT[0][:, 0:128],
                                      rhs=xn3[:, 0, 0:512],
                                      start=True, stop=True)
                if w > 0:
                    wm.ins.ldweights = False

            # ---------- qkv projections ----------
            # K/Q packed (128, n): head h lives at partitions 32h..32h+31
            # (q/k channel-major == head-major). evictions alternate DVE/ACT.
            Kp = big.tile([128, N], BF16, tag="Kp")
            Qp = big.tile([128, NH], BF16, tag="Qp")
            vt3 = big.tile([128, 32, 128], BF16, tag="vt3")
            evict_i = 0

            def evict(dst_ap, src_ap):
                nonlocal evict_i
                if evict_i % 2 == 0:
                    nc.vector.tensor_copy(dst_ap, src_ap)
                else:
                    nc.scalar.activation(dst_ap, src_ap, ACTF.Copy)
                evict_i += 1

            for g in range(4):  # K: 2 nt of 512 per slot-tile
                pk = slots.tile([128, 1024], F32, tag="slot", name=f"kp{g}")
                for half in range(2):
                    nt = 2 * g + half
                    for kc in range(2):
                        nc.tensor.matmul(pk[:, 512 * half : 512 * half + 512],
                                         lhsT=wqkvT[kc][:, 128:256],
                                         rhs=xn3[:, kc, 512 * nt : 512 * nt + 512],
                                         start=(kc == 0), stop=(kc == 1))
                evict(Kp[:, 1024 * g : 1024 * g + 1024], pk[:])
            for g in range(2):  # Q: raw q, no scale (folded into exp)
                pq = slots.tile([128, 1024], F32, tag="slot", name=f"qp{g}")
                for half in range(2):
                    nt = 2 * g + half
                    for kc in range(2):
                        nc.tensor.matmul(pq[:, 512 * half : 512 * half + 512],
                                         lhsT=wqkvT[kc][:, 0:128],
                                         rhs=xn3[:, kc, 512 * nt : 512 * nt + 512],
                                         start=(kc == 0), stop=(kc == 1))
                evict(Qp[:, 1024 * g : 1024 * g + 1024], pq[:])
            for g in range(4):  # V^T: 8 jt of (128,128) per slot-tile
                pv = slots.tile([128, 1024], F32, tag="slot", name=f"vp{g}")
                for half in range(8):
                    jt = 8 * g + half
                    for kc in range(2):
                        nc.tensor.matmul(pv[:, 128 * half : 128 * half + 128],
                                         lhsT=xn3[:, kc, 128 * jt : 128 * jt + 128],
                                         rhs=wqkvT[kc][:, 256:384],
                                         start=(kc == 0), stop=(kc == 1))
                evict(vt3[:, 8 * g : 8 * g + 8, :],
                      pv[:].rearrange("p (a b) -> p a b", a=8))

            # ---------- attention ----------
            # flat stream over 256 slots (t = s//64); attv/den pops lag by
            # ~2-3 slots and cross i-tile boundaries so the PE never drains
            # at a boundary (HAM stays warm); tail(t) is emitted as soon as
            # t's last slot is popped, interleaved with t+1's sim stream.
            def act_slot(sl):
                return (sl * ACT_SLOTS_PER_T) // 64 != ((sl + 1) * ACT_SLOTS_PER_T) // 64

            U_t, den_t = {}, {}

            def get_acc(t):
                if t not in U_t:
                    U_t[t] = pu.tile([128, 512], F32, tag="u", name=f"U{t}")
                    den_t[t] = pden.tile([128, 512], F32, tag="den",
                                         name=f"den{t}")
                return U_t[t], den_t[t]

            def attv(ex_bf, t, units):
                Ut, dent = get_acc(t)
                for q, (jc, h) in enumerate(units):
                    nc.tensor.matmul(
                        Ut[32 * h : 32 * h + 32, :],
                        lhsT=vt3[:, jc, 32 * h : 32 * h + 32],
                        rhs=ex_bf[:, 512 * q : 512 * q + 512],
                        start=(jc == 0), stop=(jc == 31),
                        tile_position=(0, 32 * h),
                        skip_group_check=True)
                for q, (jc, h) in enumerate(units):
                    nc.tensor.matmul(
                        dent[32 * h : 32 * h + 1, :],
                        lhsT=ones128x1[:],
                        rhs=ex_bf[:, 512 * q : 512 * q + 512],
                        start=(jc == 0), stop=(jc == 31),
                        tile_position=(0, 32 * h),
                        skip_group_check=True)

            attnT_t = {}

            def tail_a(t):
                # normalize chain only: frees U (for U_{t+1}) as early as
                # possible and keeps the slot ring untouched. High priority:
                # the scheduler slots these ahead of queued independent exps,
                # shortening the U_t -> U_{t+1} handover.
                Ut, dent = U_t[t], den_t[t]
                ctx = tc.high_priority(offset=40)
                ctx.__enter__()
                attnT = ptail.tile([128, 512], BF16, tag="attnT", name=f"at{t}")
                den_sb = ptail.tile([128, 512], BF16, tag="densb", name=f"dsb{t}")
                nc.scalar.activation(den_sb[:], dent[:], ACTF.Copy)
                # dp reuses the den bank (freed by the copy above)
                dp = pden.tile([128, 512], F32, tag="den", name=f"dp{t}")
                # e_full rows 32h pick den_sb row 32h and broadcast to
                # partitions 32h..32h+31 (all other rows are zero)
                nc.tensor.matmul(dp[:], lhsT=e_full[:],
                                 rhs=den_sb[:], start=True, stop=True)
                rsb = ptail.tile([128, 512], F32, tag="rsb", name=f"rsb{t}")
                nc.vector.reciprocal_approx_fast(rsb[:], dp[:])
                nc.vector.tensor_tensor(attnT[:], Ut[:], rsb[:], ALU.mult)
                ctx.__exit__(None, None, None)
                attnT_t[t] = attnT

            def tail_b(t):
                # projection + bias + store; emitted a few slots later so the
                # py matmuls never head-block the PE queue on the DVE chain.
                attnT = attnT_t[t]
                py = slots.tile([128, 1024], F32, tag="slot", name=f"py{t}")
                for kc in range(2):
                    nc.tensor.matmul(py[:, 512 * kc : 512 * kc + 512],
                                     lhsT=woT[:, 128 * kc : 128 * kc + 128],
                                     rhs=attnT[:], start=True, stop=True)
                for kc in range(2):
                    ysb = ptail.tile([128, 512], F32, tag="ysb", name=f"y{t}{kc}")
                    if kc == 0:
                        nc.scalar.activation(ysb[:], py[:, 512 * kc : 512 * kc + 512],
                                             ACTF.Identity, bias=bo_c[0][:])
                    else:
                        nc.vector.tensor_scalar_add(ysb[:], py[:, 512 * kc : 512 * kc + 512],
                                                    bo_c[1][:])
                    nc.sync.dma_start(
                        y_d[128 * kc : 128 * kc + 128, 512 * t : 512 * t + 512],
                        ysb[:])

            pending = []
            popped = [0, 0, 0, 0]
            tb_done = [False] * 4

            def pop_one():
                ex_bf, pt, punits = pending.pop(0)
                attv(ex_bf, pt, punits)
                popped[pt] += len(punits)
                if popped[pt] == 128:
                    tail_a(pt)
                if pt > 0 and popped[pt] >= 12 and not tb_done[pt - 1]:
                    tb_done[pt - 1] = True
                    tail_b(pt - 1)

            for t in range(4):
                units_all = [(jc, h) for jc in range(32) for h in range(HEADS)]
                slot_units = [units_all[i : i + 2]
                              for i in range(0, 128, 2)]
                for sl, units in enumerate(slot_units):
                    w = 512 * len(units)
                    sp = slots.tile([128, 1024], F32, tag="slot",
                                    name=f"s{t}_{sl}")
                    for q, (jc, h) in enumerate(units):
                        nc.tensor.matmul(
                            sp[:, 512 * q : 512 * q + 512],
                            lhsT=Kp[32 * h : 32 * h + 32,
                                    128 * jc : 128 * jc + 128],
                            rhs=Qp[32 * h : 32 * h + 32,
                                   512 * t : 512 * t + 512],
                            start=True, stop=True,
                            tile_position=(32 * h, 0))
                    if act_slot(sl):
                        ex = pexa.tile([128, 1024], BF16, tag="exa")
                        nc.scalar.activation(ex[:, 0:w], sp[:, 0:w], ACTF.Exp,
                                             scale=SCALE)
                        ex_bf = ex[:]
                    else:
                        exd = pexd.tile([128, 1024], I16, tag="exd")
                        nc.vector.tensor_scalar(exd[:, 0:w], sp[:, 0:w],
                                                C_SCH, D_SCH,
                                                ALU.mult, ALU.add)
                        ex_bf = exd[:].bitcast(BF16)
                    pending.append((ex_bf, t, units))
                    if sl % 2 == 1 and len(pending) >= 6:
                        pop_one()
                        pop_one()
            while pending:
                pop_one()
            tail_b(3)

    nc.compile()
    return nc


_NC_CACHE = None


def get_nc():
    global _NC_CACHE
    if _NC_CACHE is None:
        _NC_CACHE = build_nc()
    return _NC_CACHE


def shard_inputs(x, gamma, beta, w_qkv, w_out, b_out):
    """Build the 8 per-core input maps (pure slicing / layout, no math)."""
    x = np.ascontiguousarray(np.asarray(x, dtype=np.float32))
    b, c, hh, ww = x.shape
    assert (b, c, hh, ww) == (4, DIM, 64, 64)
    xf = x.reshape(b, DIM, N)
    wq = np.ascontiguousarray(np.asarray(w_qkv, dtype=np.float32))
    wo = np.ascontiguousarray(np.asarray(w_out, dtype=np.float32))
    bo = np.asarray(b_out, dtype=np.float32).reshape(DIM, 1)
    gam = np.asarray(gamma, dtype=np.float32).reshape(DIM, 1)
    bet = np.asarray(beta, dtype=np.float32).reshape(DIM, 1)
    in_maps = []
    for core in range(N_CORES):
        bi, half = core // 2, core % 2
        xr = xf[bi] if half == 0 else np.roll(xf[bi], -NH, axis=1)
        in_maps.append({
            "xr": np.ascontiguousarray(xr),
            "wq": wq, "wo": wo, "bo": bo, "gam": gam, "bet": bet,
        })
    return in_maps


def gather_outputs(per_core_y):
    """per_core_y: list of 8 arrays (256, 2048) -> (4, 256, 64, 64) f32."""
    y = np.empty((4, DIM, N), dtype=np.float32)
    for core in range(N_CORES):
        bi, half = core // 2, core % 2
        y[bi][:, NH * half : NH * half + NH] = per_core_y[core]
    return y.reshape(4, DIM, 64, 64)


def kernel(x, gamma, beta, w_qkv, w_out, b_out):
    from concourse.bass_utils import run_bass_kernel_spmd

    nc = get_nc()
    in_maps = shard_inputs(x, gamma, beta, w_qkv, w_out, b_out)
    res = run_bass_kernel_spmd(nc, in_maps, core_ids=list(range(N_CORES)))
    return gather_outputs([res.results[c]["y"] for c in range(N_CORES)])


# revision 27
# speedup vs baseline: 1.2305x; 1.0047x over previous
"""Fused GroupNorm + multi-head self-attention + output projection for
nn_Attention_55619826483814 on 8 TRN2 NeuronCores.

Reference computation (shapes hardcoded):
  x: (4, 256, 64, 64) f32
  GroupNorm(1 group) over (C,H,W) per sample -> per-channel affine (gamma, beta)
  qkv = w_qkv @ xn  (384 = 3*4heads*32dim rows)
  per head: sim = (q*scale)^T k ; attn = softmax(sim, axis=j) ; out = attn @ v
  y = w_out @ out + b_out     -> (4, 256, 64, 64)

Sharding: 8 cores = 4 batches x 2 spatial halves (identical to the previous
baseline). Core c handles batch c//2 and query positions [2048*(c%2), +2048),
with the spatial axis rolled host-side so its query half sits at columns
0..2047. Keys/values/groupnorm stats use all 4096 positions.

On-device design (v2) - the kernel is softmax-throughput bound, so both
pointwise engines stream exp concurrently while the PE runs packed matmuls:

  sim:  4 heads packed in the 128x128 PE via 4x row tiling
        (tile_position=(32h,0), K=32 each) -> one (128,512) f32 bank per
        unit (jc, h); units paired into (128,1024) PSUM slots.
  exp:  each slot is consumed by EITHER ScalarE (exact exp, scale folded)
        OR VectorE (Schraudolph: int16 <- round(sim*C + D), bitcast bf16;
        ~1.8% rms error on e^x), statically interleaved ~51/49 so both
        engines stay saturated. That beats the single-engine ACT floor
        (~295us in the baseline) by ~2x.
  attv: 4 heads packed via 4x column tiling (tile_position=(0,32h), M=32)
        accumulating U[32h:32h+32] over jc in one persistent bank;
        denominators via M=1 col-tiled ones-matmuls into a second bank.
  tail: denominator broadcast via per-quadrant indicator matmuls,
        reciprocal_approx_fast, U*(1/d) -> bf16, w_out projection, bias.

PSUM: 6 banks = 3 rotating (128,1024) slots, 1 bank U_t, 1 bank den_t.
"""

import sys

sys.path.insert(0, "/opt/trn_rl_repo")

import numpy as np

import concourse.bass as bass
import concourse.mybir as mybir
import concourse.tile as tile
from concourse import bacc
from concourse.masks import make_identity

DT = mybir.dt
F32 = DT.float32
BF16 = DT.bfloat16
I16 = DT.int16
ALU = mybir.AluOpType
ACTF = mybir.ActivationFunctionType

DIM = 256  # channels
N = 4096  # spatial positions
NH = 2048  # per-core query half
HEADS = 4
DH = 32  # head dim
HID = 128
SCALE = DH ** -0.5
EPS = 1e-5
NTOT = DIM * N

N_CORES = 8

LOG2E = 1.4426950408889634
C_SCH = 128.0 * LOG2E * SCALE  # Schraudolph scale (attn scale folded in)
D_SCH = 127.0 * 128.0 - 7.5  # bias; -7.5 centers the exp ratio error

# Of the 64 slots per i-tile, how many go to ScalarE (rest to VectorE).
ACT_SLOTS_PER_T = 35


def build_nc():
    nc = bacc.Bacc("TRN2", target_bir_lowering=False)

    xr_d = nc.dram_tensor("xr", [DIM, N], F32, kind="ExternalInput")
    wq_d = nc.dram_tensor("wq", [3 * HID, DIM], F32, kind="ExternalInput")
    wo_d = nc.dram_tensor("wo", [DIM, HID], F32, kind="ExternalInput")
    bo_d = nc.dram_tensor("bo", [DIM, 1], F32, kind="ExternalInput")
    gam_d = nc.dram_tensor("gam", [DIM, 1], F32, kind="ExternalInput")
    bet_d = nc.dram_tensor("bet", [DIM, 1], F32, kind="ExternalInput")
    y_d = nc.dram_tensor("y", [DIM, NH], F32, kind="ExternalOutput")

    with tile.TileContext(nc) as tc:
        with (
            tc.tile_pool(name="small", bufs=1) as small,
            tc.tile_pool(name="big", bufs=1) as big,
            tc.tile_pool(name="pxf", bufs=1) as pxf,
            tc.tile_pool(name="pjunk", bufs=1) as pjunk,
            tc.tile_pool(name="pwst", bufs=3) as pwst,
            tc.tile_pool(name="ptiny", bufs=2) as ptiny,
            tc.tile_pool(name="pexa", bufs=8) as pexa,
            tc.tile_pool(name="pexd", bufs=8) as pexd,
            tc.tile_pool(name="ptail", bufs=2) as ptail,
            tc.tile_pool(name="slots", bufs=3, space="PSUM") as slots,
            tc.tile_pool(name="pu", bufs=1, space="PSUM") as pu,
            tc.tile_pool(name="pden", bufs=1, space="PSUM") as pden,
        ):
            # ---------- constants ----------
            identity = small.tile([128, 128], F32, tag="ident")
            make_identity(nc, identity[:])

            ones128x32 = small.tile([128, 32], F32, tag="o12832")
            nc.gpsimd.memset(ones128x32[:], 1.0)
            ones1x128 = small.tile([1, 128], F32, tag="o1128")
            nc.gpsimd.memset(ones1x128[:], 1.0)
            ones128x1 = small.tile([128, 1], BF16, tag="o1281")
            nc.gpsimd.memset(ones128x1[:], 1.0)
            # e_full[32h, c] = (c//32 == h), used to broadcast den rows
            e_full = small.tile([128, 128], BF16, tag="efull")
            nc.gpsimd.memset(e_full[:], 0.0)
            for h in range(HEADS):
                nc.gpsimd.memset(
                    e_full[32 * h : 32 * h + 1, 32 * h : 32 * h + 32], 1.0
                )

            # ---------- load x FIRST (sync queue issues DMAs serially at
            # ~0.65us each, so x must be at the head of the queue) ----------
            xf = []
            for kc in range(2):
                t = pxf.tile([128, N], F32, tag=f"xf{kc}", name=f"xf{kc}")
                for p in range(4):
                    nc.sync.dma_start(
                        t[:, 1024 * p : 1024 * p + 1024],
                        xr_d[128 * kc : 128 * kc + 128, 1024 * p : 1024 * p + 1024])
                xf.append(t)

            gam_c, bet_c, bo_c = [], [], []
            for kc in range(2):
                g = small.tile([128, 1], F32, tag=f"gam{kc}", name=f"gam{kc}")
                nc.sync.dma_start(g[:], gam_d[128 * kc : 128 * kc + 128, :])
                gam_c.append(g)
                bt = small.tile([128, 1], F32, tag=f"bet{kc}", name=f"bet{kc}")
                nc.sync.dma_start(bt[:], bet_d[128 * kc : 128 * kc + 128, :])
                bet_c.append(bt)
                bb = small.tile([128, 1], F32, tag=f"bo{kc}", name=f"bo{kc}")
                nc.sync.dma_start(bb[:], bo_d[128 * kc : 128 * kc + 128, :])
                bo_c.append(bb)

            # ---------- weight transposes ----------
            wqkvT = [big.tile([128, 384], BF16, tag=f"wqkvT{c}", name=f"wqkvT{c}") for c in range(2)]
            for r in range(3):
                wst = pwst.tile([128, DIM], F32, tag="wst")
                nc.sync.dma_start(wst[:], wq_d[128 * r : 128 * r + 128, :])
                for c in range(2):
                    tp = slots.tile([128, 1024], F32, tag="slot", name=f"wqt{r}{c}")
                    nc.tensor.transpose(tp[:, 0:128], wst[:, 128 * c : 128 * c + 128], identity[:])
                    nc.vector.tensor_copy(wqkvT[c][:, 128 * r : 128 * r + 128], tp[:, 0:128])
            woT = big.tile([128, DIM], BF16, tag="woT")
            for r in range(2):
                wst = pwst.tile([128, HID], F32, tag="wst")
                nc.sync.dma_start(wst[:], wo_d[128 * r : 128 * r + 128, :])
                tp = slots.tile([128, 1024], F32, tag="slot", name=f"wot{r}")
                nc.tensor.transpose(tp[:, 0:128], wst[:], identity[:])
                nc.vector.tensor_copy(woT[:, 128 * r : 128 * r + 128], tp[:, 0:128])

            # ---------- groupnorm stats ----------
            # per-piece partial sums: cols 0-7 = sum(x), cols 8-15 = sum(x^2)
            st = ptiny.tile([128, 16], F32, tag="st")
            for kc in range(2):
                for p in range(4):
                    piece = xf[kc][:, 1024 * p : 1024 * p + 1024]
                    col = 4 * kc + p
                    nc.vector.reduce_sum(st[:, col : col + 1], piece,
                                         axis=mybir.AxisListType.X)
                    junk = pjunk.tile([128, 1024], BF16, tag="junk")
                    nc.scalar.activation(junk[:], piece, ACTF.Square,
                                         accum_out=st[:, 8 + col : 9 + col])
            stp = pu.tile([128, 512], F32, tag="u")
            nc.tensor.matmul(stp[0:32, 0:16], lhsT=ones128x32[:], rhs=st[:], start=True, stop=True)

            stp_sb = ptiny.tile([1, 16], F32, tag="stpsb")
            nc.vector.tensor_copy(stp_sb[:], stp[0:1, 0:16])
            mm = ptiny.tile([1, 2], F32, tag="mm")
            nc.vector.reduce_sum(mm[:], stp_sb[:].rearrange("p (a b) -> p a b", a=2),
                                 axis=mybir.AxisListType.X)
            mm2 = ptiny.tile([1, 2], F32, tag="mm2")
            nc.vector.tensor_scalar_mul(mm2[:], mm[:], 1.0 / NTOT)  # [mu, E[x^2]]
            musq = ptiny.tile([1, 1], F32, tag="musq")
            nc.vector.tensor_tensor(musq[:], mm2[:, 0:1], mm2[:, 0:1], ALU.mult)
            var = ptiny.tile([1, 1], F32, tag="var")
            nc.vector.tensor_tensor(var[:], mm2[:, 1:2], musq[:], ALU.subtract)
            vare = ptiny.tile([1, 1], F32, tag="vare")
            nc.vector.tensor_scalar_add(vare[:], var[:], EPS)
            sq = ptiny.tile([1, 1], F32, tag="sq")
            nc.scalar.activation(sq[:], vare[:], ACTF.Sqrt)
            s_t = ptiny.tile([1, 1], F32, tag="s")
            nc.vector.reciprocal(s_t[:], sq[:])
            sm = ptiny.tile([1, 2], F32, tag="sm")
            nc.vector.tensor_copy(sm[:, 0:1], s_t[:])
            nc.vector.tensor_copy(sm[:, 1:2], mm2[:, 0:1])
            bsm = pden.tile([128, 512], F32, tag="den")
            nc.tensor.matmul(bsm[:, 0:2], lhsT=ones1x128[:], rhs=sm[:], start=True, stop=True)
            a_c, d_c = [], []
            for kc in range(2):
                a = ptiny.tile([128, 1], F32, tag="ac", name=f"ac{kc}")
                nc.vector.tensor_tensor(a[:], gam_c[kc][:], bsm[:, 0:1], ALU.mult)
                a_c.append(a)
                t1 = ptiny.tile([128, 1], F32, tag="t1", name=f"t1{kc}")
                nc.vector.tensor_tensor(t1[:], a[:], bsm[:, 1:2], ALU.mult)
                d = ptiny.tile([128, 1], F32, tag="dc", name=f"dc{kc}")
                nc.vector.tensor_tensor(d[:], bet_c[kc][:], t1[:], ALU.subtract)
                d_c.append(d)

            # force the ACT exp table load now (overlaps the qkv phase)
            dummy = ptiny.tile([1, 1], F32, tag="dummy")
            nc.scalar.activation(dummy[:], vare[:], ACTF.Exp)

            # ---------- normalize + cast ----------
            xn3 = big.tile([128, 2, N], BF16, tag="xn")
            for kc in range(2):
                nc.vector.tensor_scalar(xn3[:, kc, :], xf[kc][:], a_c[kc][:],
                                        d_c[kc][:], ALU.mult, ALU.add)

            # HAM warmup: dense no-reload burst gated on xn (same gate as
            # the first qkv matmul) so the PE enters the qkv+attention stream
            # at full clock. ldweights=False keeps the array ~100% busy.
            warm = pu.tile([128, 512], F32, tag="u", name="warm")
            for w in range(12):
                wm = nc.tensor.matmul(warm[:], lhsT=wqkvT[0][:, 0:128],
                                      rhs=xn3[:, 0, 0:512],
                                      start=True, stop=True)
                if w > 0:
                    wm.ins.ldweights = False

            # ---------- qkv projections ----------
            # K/Q packed (128, n): head h lives at partitions 32h..32h+31
            # (q/k channel-major == head-major). evictions alternate DVE/ACT.
            Kp = big.tile([128, N], BF16, tag="Kp")
            Qp = big.tile([128, NH], BF16, tag="Qp")
            vt3 = big.tile([128, 32, 128], BF16, tag="vt3")
            evict_i = 0

            def evict(dst_ap, src_ap):
                nonlocal evict_i
                if evict_i % 2 == 0:
                    nc.vector.tensor_copy(dst_ap, src_ap)
                else:
                    nc.scalar.activation(dst_ap, src_ap, ACTF.Copy)
                evict_i += 1

            for g in range(4):  # K: 2 nt of 512 per slot-tile
                pk = slots.tile([128, 1024], F32, tag="slot", name=f"kp{g}")
                for half in range(2):
                    nt = 2 * g + half
                    for kc in range(2):
                        nc.tensor.matmul(pk[:, 512 * half : 512 * half + 512],
                                         lhsT=wqkvT[kc][:, 128:256],
                                         rhs=xn3[:, kc, 512 * nt : 512 * nt + 512],
                                         start=(kc == 0), stop=(kc == 1))
                evict(Kp[:, 1024 * g : 1024 * g + 1024], pk[:])
            for g in range(2):  # Q: raw q, no scale (folded into exp)
                pq = slots.tile([128, 1024], F32, tag="slot", name=f"qp{g}")
                for half in range(2):
                    nt = 2 * g + half
                    for kc in range(2):
                        nc.tensor.matmul(pq[:, 512 * half : 512 * half + 512],
                                         lhsT=wqkvT[kc][:, 0:128],
                                         rhs=xn3[:, kc, 512 * nt : 512 * nt + 512],
                                         start=(kc == 0), stop=(kc == 1))
                evict(Qp[:, 1024 * g : 1024 * g + 1024], pq[:])
            for g in range(4):  # V^T: 8 jt of (128,128) per slot-tile
                pv = slots.tile([128, 1024], F32, tag="slot", name=f"vp{g}")
                for half in range(8):
                    jt = 8 * g + half
                    for kc in range(2):
                        nc.tensor.matmul(pv[:, 128 * half : 128 * half + 128],
                                         lhsT=xn3[:, kc, 128 * jt : 128 * jt + 128],
                                         rhs=wqkvT[kc][:, 256:384],
                                         start=(kc == 0), stop=(kc == 1))
                evict(vt3[:, 8 * g : 8 * g + 8, :],
                      pv[:].rearrange("p (a b) -> p a b", a=8))

            # ---------- attention ----------
            # flat stream over 256 slots (t = s//64); attv/den pops lag by
            # ~2-3 slots and cross i-tile boundaries so the PE never drains
            # at a boundary (HAM stays warm); tail(t) is emitted as soon as
            # t's last slot is popped, interleaved with t+1's sim stream.
            def act_slot(sl):
                return (sl * ACT_SLOTS_PER_T) // 64 != ((sl + 1) * ACT_SLOTS_PER_T) // 64

            U_t, den_t = {}, {}

            def get_acc(t):
                if t not in U_t:
                    U_t[t] = pu.tile([128, 512], F32, tag="u", name=f"U{t}")
                    den_t[t] = pden.tile([128, 512], F32, tag="den",
                                         name=f"den{t}")
                return U_t[t], den_t[t]

            def attv(ex_bf, t, units):
                Ut, dent = get_acc(t)
                for q, (jc, h) in enumerate(units):
                    nc.tensor.matmul(
                        Ut[32 * h : 32 * h + 32, :],
                        lhsT=vt3[:, jc, 32 * h : 32 * h + 32],
                        rhs=ex_bf[:, 512 * q : 512 * q + 512],
                        start=(jc == 0), stop=(jc == 31),
                        tile_position=(0, 32 * h),
                        skip_group_check=True)
                for q, (jc, h) in enumerate(units):
                    nc.tensor.matmul(
                        dent[32 * h : 32 * h + 1, :],
                        lhsT=ones128x1[:],
                        rhs=ex_bf[:, 512 * q : 512 * q + 512],
                        start=(jc == 0), stop=(jc == 31),
                        tile_position=(0, 32 * h),
                        skip_group_check=True)

            attnT_t = {}

            def tail_a(t):
                # normalize chain only: frees U (for U_{t+1}) as early as
                # possible and keeps the slot ring untouched. High priority:
                # the scheduler slots these ahead of queued independent exps,
                # shortening the U_t -> U_{t+1} handover.
                Ut, dent = U_t[t], den_t[t]
                ctx = tc.high_priority(offset=40)
                ctx.__enter__()
                attnT = ptail.tile([128, 512], BF16, tag="attnT", name=f"at{t}")
                den_sb = ptail.tile([128, 512], BF16, tag="densb", name=f"dsb{t}")
                nc.scalar.activation(den_sb[:], dent[:], ACTF.Copy)
                # dp reuses the den bank (freed by the copy above)
                dp = pden.tile([128, 512], F32, tag="den", name=f"dp{t}")
                # e_full rows 32h pick den_sb row 32h and broadcast to
                # partitions 32h..32h+31 (all other rows are zero)
                nc.tensor.matmul(dp[:], lhsT=e_full[:],
                                 rhs=den_sb[:], start=True, stop=True)
                rsb = ptail.tile([128, 512], F32, tag="rsb", name=f"rsb{t}")
                nc.vector.reciprocal_approx_fast(rsb[:], dp[:])
                nc.vector.tensor_tensor(attnT[:], Ut[:], rsb[:], ALU.mult)
                ctx.__exit__(None, None, None)
                attnT_t[t] = attnT

            def tail_b(t):
                # projection + bias + store; emitted a few slots later so the
                # py matmuls never head-block the PE queue on the DVE chain.
                attnT = attnT_t[t]
                py = slots.tile([128, 1024], F32, tag="slot", name=f"py{t}")
                for kc in range(2):
                    nc.tensor.matmul(py[:, 512 * kc : 512 * kc + 512],
                                     lhsT=woT[:, 128 * kc : 128 * kc + 128],
                                     rhs=attnT[:], start=True, stop=True)
                for kc in range(2):
                    ysb = ptail.tile([128, 512], F32, tag="ysb", name=f"y{t}{kc}")
                    if kc == 0:
                        nc.scalar.activation(ysb[:], py[:, 512 * kc : 512 * kc + 512],
                                             ACTF.Identity, bias=bo_c[0][:])
                    else:
                        nc.vector.tensor_scalar_add(ysb[:], py[:, 512 * kc : 512 * kc + 512],
                                                    bo_c[1][:])
                    nc.sync.dma_start(
                        y_d[128 * kc : 128 * kc + 128, 512 * t : 512 * t + 512],
                        ysb[:])

            pending = []
            popped = [0, 0, 0, 0]
            tb_done = [False] * 4

            def pop_one():
                ex_bf, pt, punits = pending.pop(0)
                attv(ex_bf, pt, punits)
                popped[pt] += len(punits)
                if popped[pt] == 128:
                    tail_a(pt)
                if pt > 0 and popped[pt] >= 12 and not tb_done[pt - 1]:
                    tb_done[pt - 1] = True
                    tail_b(pt - 1)

            for t in range(4):
                units_all = [(jc, h) for jc in range(32) for h in range(HEADS)]
                slot_units = [units_all[i : i + 2]
                              for i in range(0, 128, 2)]
                for sl, units in enumerate(slot_units):
                    w = 512 * len(units)
                    sp = slots.tile([128, 1024], F32, tag="slot",
                                    name=f"s{t}_{sl}")
                    for q, (jc, h) in enumerate(units):
                        nc.tensor.matmul(
                            sp[:, 512 * q : 512 * q + 512],
                            lhsT=Kp[32 * h : 32 * h + 32,
                                    128 * jc : 128 * jc + 128],
                            rhs=Qp[32 * h : 32 * h + 32,
                                   512 * t : 512 * t + 512],
                            start=True, stop=True,
                            tile_position=(32 * h, 0))
                    if act_slot(sl):
                        ex = pexa.tile([128, 1024], BF16, tag="exa")
                        nc.scalar.activation(ex[:, 0:w], sp[:, 0:w], ACTF.Exp,
                                             scale=SCALE)
                        ex_bf = ex[:]
                    else:
                        exd = pexd.tile([128, 1024], I16, tag="exd")
                        nc.vector.tensor_scalar(exd[:, 0:w], sp[:, 0:w],
                                                C_SCH, D_SCH,
                                                ALU.mult, ALU.add)
                        ex_bf = exd[:].bitcast(BF16)
                    pending.append((ex_bf, t, units))
                    if sl % 2 == 1 and len(pending) >= 6:
                        pop_one()
                        pop_one()
            while pending:
                pop_one()
            tail_b(3)

    nc.compile()
    return nc


_NC_CACHE = None


def get_nc():
    global _NC_CACHE
    if _NC_CACHE is None:
        _NC_CACHE = build_nc()
    return _NC_CACHE


def shard_inputs(x, gamma, beta, w_qkv, w_out, b_out):
    """Build the 8 per-core input maps (pure slicing / layout, no math)."""
    x = np.ascontiguousarray(np.asarray(x, dtype=np.float32))
    b, c, hh, ww = x.shape
    assert (b, c, hh, ww) == (4, DIM, 64, 64)
    xf = x.reshape(b, DIM, N)
    wq = np.ascontiguousarray(np.asarray(w_qkv, dtype=np.float32))
    wo = np.ascontiguousarray(np.asarray(w_out, dtype=np.float32))
    bo = np.asarray(b_out, dtype=np.float32).reshape(DIM, 1)
    gam = np.asarray(gamma, dtype=np.float32).reshape(DIM, 1)
    bet = np.asarray(beta, dtype=np.float32).reshape(DIM, 1)
    in_maps = []
    for core in range(N_CORES):
        bi, half = core // 2, core % 2
        xr = xf[bi] if half == 0 else np.roll(xf[bi], -NH, axis=1)
        in_maps.append({
            "xr": np.ascontiguousarray(xr),
            "wq": wq, "wo": wo, "bo": bo, "gam": gam, "bet": bet,
        })
    return in_maps


def gather_outputs(per_core_y):
    """per_core_y: list of 8 arrays (256, 2048) -> (4, 256, 64, 64) f32."""
    y = np.empty((4, DIM, N), dtype=np.float32)
    for core in range(N_CORES):
        bi, half = core // 2, core % 2
        y[bi][:, NH * half : NH * half + NH] = per_core_y[core]
    return y.reshape(4, DIM, 64, 64)


def kernel(x, gamma, beta, w_qkv, w_out, b_out):
    from concourse.bass_utils import run_bass_kernel_spmd

    nc = get_nc()
    in_maps = shard_inputs(x, gamma, beta, w_qkv, w_out, b_out)
    res = run_bass_kernel_spmd(nc, in_maps, core_ids=list(range(N_CORES)))
    return gather_outputs([res.results[c]["y"] for c in range(N_CORES)])
